# revision 16
# baseline (speedup 1.0000x reference)
"""Trainium2 Bass kernel for nn_AttentionBlock (b=16, c=32, 128x128 spatial,
heads=8, dim_head=64).

Sharding: 4 spatial shards x 2 batch groups across 8 NeuronCores, with the
Gram matrix S = X X^T computed REDUNDANTLY on every core over the full
group N=16384 (instead of partial-S + AllReduce). The ncfw collective path
costs ~81us serial on this runtime (46us barrier + trigger pickup + 2 ARs)
and re-throttles the PE HAM clock during the idle wait; replicating S costs
~25us of extra PE streaming and removes all cross-core communication.

Algebraic structure (per core, X = group x reshaped [256=(8b x 32c), 16384]):
  S    = X X^T                  (Gram matrix, [256, 256]; m-reduction on PE;
                                 only S[0:128,:] and S[128:,128:] computed,
                                 S[128:,0:128] restored by PE transpose)
  sim_h = Wq_h (S/8) Wk_h^T     (tiny; q/k never materialized)
  attn = softmax(sim)           (local -- S is complete, no reduce needed)
  At[f=(h,j), o] = sum_i attn[h][i, j] Wout[o, h*64+i]
  Mt[c, o] = sum_f Wv[f, c] At[f, o]
  out[o, m] = sum_c Mt[c, o] X[c, m] + bout[o]   (own m-shard of 4096 only)

Perf notes: xT pieces go down the two HWDGE queues (sync/scalar, first
descriptor ~7us after NEFF start; gpsimd SWDGE costs ~1.4us/dma_start and
doesn't fire until ~11us, so it only carries the late-needed x/weights).
A dummy matmul keyed on the exp() output keeps the PE HAM clock at 8/8
through the softmax gap so the final gemm runs at 2.4GHz. Output is stored
bf16 (host upcasts) to halve the drain.
"""

import numpy as np

N_CORES = 8
B, C, HS, WS = 16, 32, 128, 128
N = HS * WS              # 16384
NSH = N // 4             # 4096 per-core spatial shard
H = 8
DH = 64
QD = H * C               # 256
INNER = H * DH           # 512
SCALE = DH ** -0.5       # 0.125
MTF = N // 128           # 128 m-chunks of 128 over the FULL group
NWARM = 40               # PE prewarm dummy matmuls (span the DMA lead time)
# xT piece schedule: (start_chunk, n_chunks, engine_idx) with engines
# [sync, scalar, gpsimd]. Small 4-chunk starters on the two HWDGE queues
# start S early; 1MB body pieces (8KB/partition descriptors -- the DMA
# engines' per-descriptor cost makes 8KB ~1.5x the GB/s of 4KB) are
# balanced so each queue's pieces land just ahead of S's consumption.
XT_PIECES = [
    (0, 4, 0), (4, 4, 1), (8, 4, 0), (12, 4, 1),
    (16, 16, 0), (32, 16, 1), (48, 16, 2),
    (64, 16, 0), (80, 16, 1), (96, 16, 2),
    (112, 8, 0), (120, 8, 1),
]

_CACHE = {}


def _build_nc():
    import concourse.bacc as bacc
    import concourse.mybir as mybir
    import concourse.tile as tile
    from concourse.masks import make_identity
    from contextlib import ExitStack

    f32 = mybir.dt.float32
    bf16 = mybir.dt.bfloat16

    nc = bacc.Bacc("TRN2", target_bir_lowering=False, debug=False,
                   num_devices=N_CORES)

    # all inputs partition-major, pre-cast to bf16 on host so every DMA
    # descriptor is >=4KB contiguous and no cast-DMA is needed
    xt_ext = nc.dram_tensor("xT", [128, MTF, QD], bf16, kind="ExternalInput")
    x_ext = nc.dram_tensor("x", [128, 2, NSH], bf16, kind="ExternalInput")
    wq_ext = nc.dram_tensor("Wq", [128, 2, INNER], bf16, kind="ExternalInput")
    wk_ext = nc.dram_tensor("Wk", [128, 2, INNER], bf16, kind="ExternalInput")
    wv_ext = nc.dram_tensor("Wv", [128, 4, QD], bf16, kind="ExternalInput")
    wo_ext = nc.dram_tensor("Wout", [128, 4, QD], bf16, kind="ExternalInput")
    bout_ext = nc.dram_tensor("bout", [128, 2], f32, kind="ExternalInput")
    out_ext = nc.dram_tensor("out", [2, 128, NSH], bf16, kind="ExternalOutput")

    with tile.TileContext(nc) as tc:
        with ExitStack() as ctx:
            persist = ctx.enter_context(tc.tile_pool(name="persist", bufs=1))

            # identity first on gpsimd: ready ~6us, feeds the PE prewarm
            ident = persist.tile([128, 128], bf16, tag="ident")
            make_identity(nc, ident[:])

            # ---- loads: xT pieces per XT_PIECES over all 3 DMA queues;
            # weights ride behind gpsimd's pieces, x behind sync/scalar's.
            xt = persist.tile([128, MTF, QD], bf16, tag="xt")
            engs = [nc.sync, nc.scalar, nc.gpsimd]
            for (c0, nch, ei) in XT_PIECES:
                engs[ei].dma_start(
                    xt[:, c0:c0 + nch, :], xt_ext[:, c0:c0 + nch, :])

            wqT = persist.tile([128, 2, INNER], bf16, tag="wqT")
            nc.gpsimd.dma_start(wqT[:], wq_ext[:])
            wkT = persist.tile([128, 2, INNER], bf16, tag="wkT")
            nc.gpsimd.dma_start(wkT[:], wk_ext[:])
            wv_bf = persist.tile([128, 4, QD], bf16, tag="wv_bf")
            nc.gpsimd.dma_start(wv_bf[:], wv_ext[:])
            woT = persist.tile([128, 4, QD], bf16, tag="woT")
            nc.gpsimd.dma_start(woT[:], wo_ext[:])
            bout_sb = persist.tile([128, 2], f32, tag="bout_sb")
            nc.gpsimd.dma_start(bout_sb[:], bout_ext[:])
            x_bf = persist.tile([128, 2, NSH], bf16, tag="x_bf")
            for cc in range(2):
                eng = nc.sync if cc == 0 else nc.scalar
                eng.dma_start(x_bf[:, cc, :], x_ext[:, cc, :])

            S_bf = [persist.tile([128, 256], bf16, tag=f"S{c}",
                                 name=f"S{c}") for c in range(2)]
            T1 = [persist.tile([128, 512], bf16, tag=f"T1{c}",
                               name=f"T1{c}") for c in range(2)]

            with tc.tile_pool(name="S0ps", bufs=1, space="PSUM") as S0pool, \
                 tc.tile_pool(name="S11ps", bufs=1, space="PSUM") as S11pool, \
                 tc.tile_pool(name="tpps", bufs=1, space="PSUM") as tppool, \
                 tc.tile_pool(name="t1ps", bufs=2, space="PSUM") as t1ps, \
                 tc.tile_pool(name="wrm1", bufs=1, space="PSUM") as wrm1, \
                 tc.tile_pool(name="simps", bufs=1, space="PSUM") as simpool:

                # PE prewarm: dummy matmuls on the identity tile bridge the
                # ~5us DMA lead time so the HAM clock is already 8/8 (2.4GHz)
                # when the first xT piece lands
                wm = wrm1.tile([128, 128], f32, tag="wm")
                for _ in range(NWARM):
                    nc.tensor.matmul(wm[:], ident[:], ident[:],
                                     start=True, stop=True)

                # S = X X^T over the FULL group m range. Symmetric: compute
                # rows 0-127 x all cols (S0) and the S11 block; S10 = S01^T.
                # Two accumulation groups in two separate PSUM banks.
                S0_ps = S0pool.tile([128, 256], f32, tag="S0ps")
                S11_ps = S11pool.tile([128, 128], f32, tag="S11ps")
                for mc in range(MTF):
                    nc.tensor.matmul(
                        S0_ps[:], xt[:, mc, 0:128], xt[:, mc, :],
                        start=(mc == 0), stop=(mc == MTF - 1))
                    nc.tensor.matmul(
                        S11_ps[:], xt[:, mc, 128:256], xt[:, mc, 128:256],
                        start=(mc == 0), stop=(mc == MTF - 1))
                nc.vector.tensor_copy(S_bf[0][:], S0_ps[:])
                nc.scalar.copy(S_bf[1][:, 128:256], S11_ps[:])
                tp_ps = tppool.tile([128, 128], bf16, tag="tpps")
                nc.tensor.transpose(tp_ps[:], S_bf[0][:, 128:256], ident[:])
                nc.vector.tensor_copy(S_bf[1][:, 0:128], tp_ps[:])

                # T1 = S Wk^T  (S symmetric; accumulate over c2 chunks)
                for c1 in range(2):
                    t1p = t1ps.tile([128, 512], f32, tag="t1p")
                    for c2 in range(2):
                        nc.tensor.matmul(
                            t1p[:],
                            S_bf[c2][:, c1 * 128:(c1 + 1) * 128],
                            wkT[:, c2, :],
                            start=(c2 == 0), stop=(c2 == 1))
                    if c1 == 0:
                        nc.vector.tensor_copy(T1[c1][:], t1p[:])
                    else:
                        nc.scalar.copy(T1[c1][:], t1p[:])

                # sim_h = Wq_h T1_h  (diagonal 128-blocks)
                sim_ps = simpool.tile([128, 512], f32, tag="simps")
                for hp in range(4):
                    hs = slice(hp * 128, (hp + 1) * 128)
                    for c1 in range(2):
                        nc.tensor.matmul(
                            sim_ps[:, hs], wqT[:, c1, hs], T1[c1][:, hs],
                            start=(hp == 0 and c1 == 0),
                            stop=(hp == 3 and c1 == 1))

                # fused extract+softmax-numerator: expt = exp(SCALE*sim)
                # straight from PSUM, with the row-sum accumulated for free
                # (logits are O(0.5) so exp needs no max-shift). Separate
                # tiles per diagonal block hp so downstream consumers (and
                # the PE keep-warm matmul) fire per-block instead of waiting
                # for the whole extract phase.
                expt = [persist.tile([128, 64], f32, tag=f"expt{hp}",
                                     name=f"expt{hp}") for hp in range(4)]
                sums = [persist.tile([128, 1], f32, tag=f"sums{hp}",
                                     name=f"sums{hp}") for hp in range(4)]
                Exp = mybir.ActivationFunctionType.Exp
                for hp in range(4):
                    nc.scalar.activation(
                        expt[hp][0:64, :],
                        sim_ps[0:64, hp * 128:hp * 128 + 64], Exp,
                        scale=SCALE, accum_out=sums[hp][0:64, :])
                    nc.scalar.activation(
                        expt[hp][64:128, :],
                        sim_ps[64:128, hp * 128 + 64:hp * 128 + 128], Exp,
                        scale=SCALE, accum_out=sums[hp][64:128, :])

            # ---- softmax -> At -> Mt -> final gemm -> out stores ----
            with tc.tile_pool(name="smx", bufs=1) as smx, \
                 tc.tile_pool(name="warm", bufs=1, space="PSUM") as warmpool, \
                 tc.tile_pool(name="aps", bufs=2, space="PSUM") as aps, \
                 tc.tile_pool(name="mps", bufs=1, space="PSUM") as mps, \
                 tc.tile_pool(name="ops", bufs=4, space="PSUM") as ops:
                At = [persist.tile([128, 256], bf16, tag=f"At{fc}",
                                   name=f"At{fc}") for fc in range(4)]
                Mt = [persist.tile([128, 256], bf16, tag=f"Mt{cc}",
                                   name=f"Mt{cc}") for cc in range(2)]
                out_sb = persist.tile([128, 2, NSH], bf16, tag="out_sb")

                # dummy matmul keyed on the FIRST exp block only: fires
                # ~1us into the softmax chain and keeps the PE HAM activity
                # window busy so the tail doesn't drop to the 1.2GHz clock
                warm_ps = warmpool.tile([128, 64], f32, tag="warm")
                nc.tensor.matmul(warm_ps[0:64, :], expt[0][:],
                                 expt[0][:], start=True, stop=True)

                # per-block softmax finish + At, pipelined across engines:
                # recip(hp) -> attn(hp) -> At matmuls(hp) while hp+1's
                # extract is still running on the scalar engine.
                # At[fc][(parity,j), o] = sum_i attn[h][i, j] WoutT[f, o]
                for fc in range(4):
                    rsum = smx.tile([128, 1], f32, tag=f"rsum{fc}",
                                    name=f"rsum{fc}")
                    nc.vector.reciprocal(rsum[:], sums[fc][:])
                    attn_bf = smx.tile([128, 64], bf16, tag=f"attn{fc}",
                                       name=f"attn{fc}")
                    nc.vector.tensor_mul(attn_bf[:], expt[fc][:],
                                         rsum[:].broadcast_to([128, 64]))
                    ap_t = aps.tile([128, 256], f32, tag="ap_t")
                    for parity in range(2):
                        ps = slice(parity * 64, (parity + 1) * 64)
                        nc.tensor.matmul(ap_t[ps, :], attn_bf[ps, :],
                                         woT[ps, fc, :],
                                         start=True, stop=True)
                    if fc % 2 == 0:
                        nc.vector.tensor_copy(At[fc][:], ap_t[:])
                    else:
                        nc.scalar.copy(At[fc][:], ap_t[:])

                # Mt[c, o] = sum_f Wv[f, c] At[f, o]
                mp = mps.tile([128, 512], f32, tag="mp")
                for cchunk in range(2):
                    cs = slice(cchunk * 128, (cchunk + 1) * 128)
                    for fc in range(4):
                        nc.tensor.matmul(
                            mp[:, cchunk * 256:(cchunk + 1) * 256],
                            wv_bf[:, fc, cs], At[fc][:],
                            start=(cchunk == 0 and fc == 0),
                            stop=(cchunk == 1 and fc == 3))
                for cchunk in range(2):
                    eng_copy = (nc.vector.tensor_copy if cchunk == 0
                                else nc.scalar.copy)
                    eng_copy(Mt[cchunk][:],
                             mp[:, cchunk * 256:(cchunk + 1) * 256])

                # out[o, m] = sum_c Mt[c, o] X[c, m] + bout; store per 1024-m
                for ot in range(2):
                    os_ = slice(ot * 128, (ot + 1) * 128)
                    for pc in range(8):
                        op_t = ops.tile([128, 512], f32, tag="op_t")
                        for cc in range(2):
                            nc.tensor.matmul(op_t[:], Mt[cc][:, os_],
                                             x_bf[:, cc, pc * 512:(pc + 1) * 512],
                                             start=(cc == 0), stop=(cc == 1))
                        dst = out_sb[:, ot, pc * 512:(pc + 1) * 512]
                        if pc % 2 == 0:
                            nc.vector.tensor_scalar_add(
                                dst, op_t[:], bout_sb[:, ot:ot + 1])
                        else:
                            nc.scalar.activation(
                                dst, op_t[:],
                                mybir.ActivationFunctionType.Identity,
                                bias=bout_sb[:, ot:ot + 1])
                            q = pc // 2
                            nc.sync.dma_start(
                                out_ext[ot, :, q * 1024:(q + 1) * 1024],
                                out_sb[:, ot, q * 1024:(q + 1) * 1024])

    nc.compile()
    return nc


def _get_nc():
    if "nc" not in _CACHE:
        _CACHE["nc"] = _build_nc()
    return _CACHE["nc"]


def make_in_maps(x, Wq, Wkv, Wout, bout):
    import ml_dtypes
    bf16 = ml_dtypes.bfloat16
    xf = np.asarray(x, dtype=np.float32).reshape(B, C, N)
    Wq_r = np.asarray(Wq, np.float32).T.reshape(2, 128, INNER)
    Wk_r = np.asarray(Wkv, np.float32)[:INNER].T.reshape(2, 128, INNER)
    Wv_r = np.asarray(Wkv, np.float32)[INNER:].reshape(4, 128, QD)
    Wo_r = np.asarray(Wout, np.float32).T.reshape(4, 128, QD)
    bout_r = np.ascontiguousarray(
        np.asarray(bout, np.float32).reshape(2, 128).transpose(1, 0))
    Wq_r = Wq_r.transpose(1, 0, 2).astype(bf16)    # [128, 2, 512]
    Wk_r = Wk_r.transpose(1, 0, 2).astype(bf16)
    Wv_r = Wv_r.transpose(1, 0, 2).astype(bf16)    # [128, 4, 256]
    Wo_r = Wo_r.transpose(1, 0, 2).astype(bf16)
    maps = []
    xt_groups = []
    for g in range(2):
        Xg = xf[g * 8:(g + 1) * 8].reshape(QD, N)
        # [128 part = m%128, mc, 256 f] m-major full-group transpose
        xt_groups.append(
            Xg.T.reshape(MTF, 128, QD).transpose(1, 0, 2).astype(bf16))
    for i in range(N_CORES):
        g, s = divmod(i, 4)
        Xg = xf[g * 8:(g + 1) * 8].reshape(QD, N)
        xs = Xg[:, s * NSH:(s + 1) * NSH]
        # [(4b x 32c) part, cc, m] natural shard
        xs_n = xs.reshape(2, 128, NSH).transpose(1, 0, 2).astype(bf16)
        maps.append({
            "xT": xt_groups[g], "x": xs_n,
            "Wq": Wq_r, "Wk": Wk_r, "Wv": Wv_r, "Wout": Wo_r,
            "bout": bout_r,
        })
    return maps


def gather_out(results):
    out = np.empty((B, C, N), dtype=np.float32)
    for i in range(N_CORES):
        g, s = divmod(i, 4)
        r = np.asarray(results[i]["out"], np.float32).reshape(2, 4, C, NSH)
        for ot in range(2):
            out[g * 8 + ot * 4:g * 8 + (ot + 1) * 4, :,
                s * NSH:(s + 1) * NSH] = r[ot]
    return out.reshape(B, C, HS, WS)


def run_sharded(in_maps, **kw):
    from concourse.bass_utils import run_bass_kernel_spmd
    nc = _get_nc()
    return run_bass_kernel_spmd(nc, in_maps, list(range(N_CORES)), **kw)


def kernel(x, Wq, Wkv, Wout, bout):
    in_maps = make_in_maps(x, Wq, Wkv, Wout, bout)
    res = run_sharded(in_maps)
    return gather_out(res.results)


if __name__ == "__main__":
    nc = _get_nc()
    print("built + compiled OK")


# revision 23
# speedup vs baseline: 1.0316x; 1.0316x over previous
"""Trainium2 Bass kernel for nn_AttentionBlock (b=16, c=32, 128x128 spatial,
heads=8, dim_head=64).

Sharding: 4 spatial shards x 2 batch groups across 8 NeuronCores, with the
Gram matrix S = X X^T computed REDUNDANTLY on every core over the full
group N=16384 (instead of partial-S + AllReduce). The ncfw collective path
costs ~81us serial on this runtime (46us barrier + trigger pickup + 2 ARs)
and re-throttles the PE HAM clock during the idle wait; replicating S costs
~25us of extra PE streaming and removes all cross-core communication.

Algebraic structure (per core, X = group x reshaped [256=(8b x 32c), 16384]):
  S    = X X^T                  (Gram matrix, [256, 256]; m-reduction on PE;
                                 only S[0:128,:] and S[128:,128:] computed,
                                 S[128:,0:128] restored by PE transpose)
  sim_h = Wq_h (S/8) Wk_h^T     (tiny; q/k never materialized)
  attn = softmax(sim)           (local -- S is complete, no reduce needed)
  At[f=(h,j), o] = sum_i attn[h][i, j] Wout[o, h*64+i]
  Mt[c, o] = sum_f Wv[f, c] At[f, o]
  out[o, m] = sum_c Mt[c, o] X[c, m] + bout[o]   (own m-shard of 4096 only)

Perf notes: xT pieces go down the two HWDGE queues (sync/scalar, first
descriptor ~7us after NEFF start; gpsimd SWDGE costs ~1.4us/dma_start and
doesn't fire until ~11us, so it only carries the late-needed x/weights).
A dummy matmul keyed on the exp() output keeps the PE HAM clock at 8/8
through the softmax gap so the final gemm runs at 2.4GHz. Output is stored
bf16 (host upcasts) to halve the drain.
"""

import numpy as np

N_CORES = 8
B, C, HS, WS = 16, 32, 128, 128
N = HS * WS              # 16384
NSH = N // 4             # 4096 per-core spatial shard
H = 8
DH = 64
QD = H * C               # 256
INNER = H * DH           # 512
SCALE = DH ** -0.5       # 0.125
MTF = N // 128           # 128 m-chunks of 128 over the FULL group
NWARM = 40               # PE prewarm dummy matmuls (span the DMA lead time)
# xT piece schedule: (start_chunk, n_chunks, engine_idx) with engines
# [sync, scalar, gpsimd]. 512KB pieces (4KB/partition descriptors) keep
# per-piece latency ~3.5us; sync/scalar (first descriptor ~8.5us) carry
# the early pieces, gpsimd (SWDGE, first descriptor ~11.6us) carries
# pieces the PE only needs 16+ chunks in, so no queue's delivery lags
# S's ~1.4us/piece consumption.
XT_PIECES = [
    (0, 8, 0), (8, 8, 1), (16, 8, 0), (24, 8, 1), (32, 8, 2),
    (40, 8, 0), (48, 8, 1), (56, 8, 2), (64, 8, 0), (72, 8, 1),
    (80, 8, 2), (88, 8, 0), (96, 8, 1), (104, 8, 2), (112, 8, 0),
    (120, 8, 1),
]

_CACHE = {}


def _build_nc():
    import concourse.bacc as bacc
    import concourse.mybir as mybir
    import concourse.tile as tile
    from concourse.masks import make_identity
    from contextlib import ExitStack

    f32 = mybir.dt.float32
    bf16 = mybir.dt.bfloat16

    nc = bacc.Bacc("TRN2", target_bir_lowering=False, debug=False,
                   num_devices=N_CORES)

    # all inputs partition-major, pre-cast to bf16 on host so every DMA
    # descriptor is >=4KB contiguous and no cast-DMA is needed
    xt_ext = nc.dram_tensor("xT", [128, MTF, QD], bf16, kind="ExternalInput")
    x_ext = nc.dram_tensor("x", [128, 2, NSH], bf16, kind="ExternalInput")
    wq_ext = nc.dram_tensor("Wq", [128, 2, INNER], bf16, kind="ExternalInput")
    wk_ext = nc.dram_tensor("Wk", [128, 2, INNER], bf16, kind="ExternalInput")
    wv_ext = nc.dram_tensor("Wv", [128, 4, QD], bf16, kind="ExternalInput")
    wo_ext = nc.dram_tensor("Wout", [128, 4, QD], bf16, kind="ExternalInput")
    bout_ext = nc.dram_tensor("bout", [128, 2], f32, kind="ExternalInput")
    out_ext = nc.dram_tensor("out", [2, 128, NSH], bf16, kind="ExternalOutput")

    with tile.TileContext(nc) as tc:
        with ExitStack() as ctx:
            persist = ctx.enter_context(tc.tile_pool(name="persist", bufs=1))

            # identity first on gpsimd: ready ~6us, feeds the PE prewarm
            ident = persist.tile([128, 128], bf16, tag="ident")
            make_identity(nc, ident[:])

            # ---- loads: xT pieces per XT_PIECES over all 3 DMA queues;
            # weights ride behind gpsimd's pieces, x behind sync/scalar's.
            xt = persist.tile([128, MTF, QD], bf16, tag="xt")
            engs = [nc.sync, nc.scalar, nc.gpsimd]
            for (c0, nch, ei) in XT_PIECES:
                engs[ei].dma_start(
                    xt[:, c0:c0 + nch, :], xt_ext[:, c0:c0 + nch, :])

            wqT = persist.tile([128, 2, INNER], bf16, tag="wqT")
            nc.gpsimd.dma_start(wqT[:], wq_ext[:])
            wkT = persist.tile([128, 2, INNER], bf16, tag="wkT")
            nc.gpsimd.dma_start(wkT[:], wk_ext[:])
            wv_bf = persist.tile([128, 4, QD], bf16, tag="wv_bf")
            nc.gpsimd.dma_start(wv_bf[:], wv_ext[:])
            woT = persist.tile([128, 4, QD], bf16, tag="woT")
            nc.gpsimd.dma_start(woT[:], wo_ext[:])
            bout_sb = persist.tile([128, 2], f32, tag="bout_sb")
            nc.gpsimd.dma_start(bout_sb[:], bout_ext[:])
            x_bf = persist.tile([128, 2, NSH], bf16, tag="x_bf")
            for cc in range(2):
                for hh in range(2):
                    eng = nc.sync if (cc + hh) % 2 == 0 else nc.scalar
                    eng.dma_start(
                        x_bf[:, cc, hh * 2048:(hh + 1) * 2048],
                        x_ext[:, cc, hh * 2048:(hh + 1) * 2048])

            S_bf = [persist.tile([128, 256], bf16, tag=f"S{c}",
                                 name=f"S{c}") for c in range(2)]
            T1 = [persist.tile([128, 512], bf16, tag=f"T1{c}",
                               name=f"T1{c}") for c in range(2)]

            with tc.tile_pool(name="S0ps", bufs=1, space="PSUM") as S0pool, \
                 tc.tile_pool(name="S11ps", bufs=1, space="PSUM") as S11pool, \
                 tc.tile_pool(name="tpps", bufs=1, space="PSUM") as tppool, \
                 tc.tile_pool(name="t1ps", bufs=2, space="PSUM") as t1ps, \
                 tc.tile_pool(name="wrm1", bufs=1, space="PSUM") as wrm1, \
                 tc.tile_pool(name="simps", bufs=1, space="PSUM") as simpool:

                # PE prewarm: dummy matmuls on the identity tile bridge the
                # ~5us DMA lead time so the HAM clock is already 8/8 (2.4GHz)
                # when the first xT piece lands
                wm = wrm1.tile([128, 256], f32, tag="wm")
                for _ in range(NWARM):
                    nc.tensor.matmul(wm[:, 0:128], ident[:], ident[:],
                                     start=True, stop=True)

                # S = X X^T over the FULL group m range. Symmetric: compute
                # rows 0-127 x all cols (S0) and the S11 block; S10 = S01^T.
                # Two accumulation groups in two separate PSUM banks.
                S0_ps = S0pool.tile([128, 256], f32, tag="S0ps")
                S11_ps = S11pool.tile([128, 128], f32, tag="S11ps")
                for mc in range(MTF):
                    nc.tensor.matmul(
                        S0_ps[:], xt[:, mc, 0:128], xt[:, mc, :],
                        start=(mc == 0), stop=(mc == MTF - 1))
                    nc.tensor.matmul(
                        S11_ps[:], xt[:, mc, 128:256], xt[:, mc, 128:256],
                        start=(mc == 0), stop=(mc == MTF - 1))
                nc.vector.tensor_copy(S_bf[0][:], S0_ps[:])
                nc.scalar.copy(S_bf[1][:, 128:256], S11_ps[:])
                tp_ps = tppool.tile([128, 128], bf16, tag="tpps")
                nc.tensor.transpose(tp_ps[:], S_bf[0][:, 128:256], ident[:])
                # junk burst keyed on S_bf[0]: keeps PE activity dense while
                # the S10 reconstruction copies run (HAM re-throttles on low
                # duty, not just full idle)
                for _ in range(4):
                    nc.tensor.matmul(wm[:, 128:256], S_bf[0][:, 0:128],
                                     S_bf[0][:, 0:128], start=True, stop=True)
                nc.vector.tensor_copy(S_bf[1][:, 0:128], tp_ps[:])

                # T1 = S Wk^T  (S symmetric; accumulate over c2 chunks)
                for c1 in range(2):
                    t1p = t1ps.tile([128, 512], f32, tag="t1p")
                    for c2 in range(2):
                        nc.tensor.matmul(
                            t1p[:],
                            S_bf[c2][:, c1 * 128:(c1 + 1) * 128],
                            wkT[:, c2, :],
                            start=(c2 == 0), stop=(c2 == 1))
                    if c1 == 0:
                        nc.vector.tensor_copy(T1[c1][:], t1p[:])
                    else:
                        nc.scalar.copy(T1[c1][:], t1p[:])

                # sim_h = Wq_h T1_h  (diagonal 128-blocks)
                sim_ps = simpool.tile([128, 512], f32, tag="simps")
                for hp in range(4):
                    hs = slice(hp * 128, (hp + 1) * 128)
                    for c1 in range(2):
                        nc.tensor.matmul(
                            sim_ps[:, hs], wqT[:, c1, hs], T1[c1][:, hs],
                            start=(hp == 0 and c1 == 0),
                            stop=(hp == 3 and c1 == 1))

                # fused extract+softmax-numerator: expt = exp(SCALE*sim)
                # straight from PSUM, with the row-sum accumulated for free
                # (logits are O(0.5) so exp needs no max-shift). Separate
                # tiles per diagonal block hp so downstream consumers (and
                # the PE keep-warm matmul) fire per-block instead of waiting
                # for the whole extract phase.
                expt = [persist.tile([128, 64], f32, tag=f"expt{hp}",
                                     name=f"expt{hp}") for hp in range(4)]
                sums = [persist.tile([128, 1], f32, tag=f"sums{hp}",
                                     name=f"sums{hp}") for hp in range(4)]
                Exp = mybir.ActivationFunctionType.Exp
                for hp in range(4):
                    nc.scalar.activation(
                        expt[hp][0:64, :],
                        sim_ps[0:64, hp * 128:hp * 128 + 64], Exp,
                        scale=SCALE, accum_out=sums[hp][0:64, :])
                    nc.scalar.activation(
                        expt[hp][64:128, :],
                        sim_ps[64:128, hp * 128 + 64:hp * 128 + 128], Exp,
                        scale=SCALE, accum_out=sums[hp][64:128, :])

            # ---- softmax -> At -> Mt -> final gemm -> out stores ----
            with tc.tile_pool(name="smx", bufs=1) as smx, \
                 tc.tile_pool(name="warm", bufs=1, space="PSUM") as warmpool, \
                 tc.tile_pool(name="aps", bufs=2, space="PSUM") as aps, \
                 tc.tile_pool(name="mps", bufs=1, space="PSUM") as mps, \
                 tc.tile_pool(name="ops", bufs=3, space="PSUM") as ops:
                At = [persist.tile([128, 256], bf16, tag=f"At{fc}",
                                   name=f"At{fc}") for fc in range(4)]
                Mt = [persist.tile([128, 256], bf16, tag=f"Mt{cc}",
                                   name=f"Mt{cc}") for cc in range(2)]
                out_sb = persist.tile([128, 2, NSH], bf16, tag="out_sb")

                # junk bursts keyed on successive chain tiles: each fires as
                # its tile is produced, keeping the PE HAM activity window
                # dense through the softmax chain so the tail stays at 2.4GHz
                warm_ps = warmpool.tile([128, 64], f32, tag="warm")
                warm_b = warmpool.tile([128, 256], f32, tag="warmb")
                for _ in range(4):
                    nc.tensor.matmul(warm_ps[0:64, :], expt[0][:],
                                     expt[0][:], start=True, stop=True)
                for _ in range(4):
                    nc.tensor.matmul(warm_ps[0:64, :], expt[2][:],
                                     expt[2][:], start=True, stop=True)

                # per-block softmax finish + At, pipelined across engines:
                # recip(hp) -> attn(hp) -> At matmuls(hp) while hp+1's
                # extract is still running on the scalar engine.
                # At[fc][(parity,j), o] = sum_i attn[h][i, j] WoutT[f, o]
                for fc in range(4):
                    rsum = smx.tile([128, 1], f32, tag=f"rsum{fc}",
                                    name=f"rsum{fc}")
                    nc.vector.reciprocal(rsum[:], sums[fc][:])
                    attn_bf = smx.tile([128, 64], bf16, tag=f"attn{fc}",
                                       name=f"attn{fc}")
                    nc.vector.tensor_mul(attn_bf[:], expt[fc][:],
                                         rsum[:].broadcast_to([128, 64]))
                    ap_t = aps.tile([128, 256], f32, tag="ap_t")
                    for parity in range(2):
                        ps = slice(parity * 64, (parity + 1) * 64)
                        nc.tensor.matmul(ap_t[ps, :], attn_bf[ps, :],
                                         woT[ps, fc, :],
                                         start=True, stop=True)
                    if fc % 2 == 0:
                        nc.vector.tensor_copy(At[fc][:], ap_t[:])
                    else:
                        nc.scalar.copy(At[fc][:], ap_t[:])
                    if fc == 1:
                        for _ in range(4):
                            nc.tensor.matmul(warm_b[:], At[0][:, 0:128],
                                             At[0][:], start=True, stop=True)

                # Mt[c, o] = sum_f Wv[f, c] At[f, o]
                mp = mps.tile([128, 512], f32, tag="mp")
                for cchunk in range(2):
                    cs = slice(cchunk * 128, (cchunk + 1) * 128)
                    for fc in range(4):
                        nc.tensor.matmul(
                            mp[:, cchunk * 256:(cchunk + 1) * 256],
                            wv_bf[:, fc, cs], At[fc][:],
                            start=(cchunk == 0 and fc == 0),
                            stop=(cchunk == 1 and fc == 3))
                for cchunk in range(2):
                    eng_copy = (nc.vector.tensor_copy if cchunk == 0
                                else nc.scalar.copy)
                    eng_copy(Mt[cchunk][:],
                             mp[:, cchunk * 256:(cchunk + 1) * 256])

                # out[o, m] = sum_c Mt[c, o] X[c, m] + bout; store per 1024-m
                for ot in range(2):
                    os_ = slice(ot * 128, (ot + 1) * 128)
                    for pc in range(8):
                        op_t = ops.tile([128, 512], f32, tag="op_t")
                        for cc in range(2):
                            nc.tensor.matmul(op_t[:], Mt[cc][:, os_],
                                             x_bf[:, cc, pc * 512:(pc + 1) * 512],
                                             start=(cc == 0), stop=(cc == 1))
                        dst = out_sb[:, ot, pc * 512:(pc + 1) * 512]
                        if pc % 2 == 0:
                            nc.vector.tensor_scalar_add(
                                dst, op_t[:], bout_sb[:, ot:ot + 1])
                        else:
                            nc.scalar.activation(
                                dst, op_t[:],
                                mybir.ActivationFunctionType.Identity,
                                bias=bout_sb[:, ot:ot + 1])
                            q = pc // 2
                            nc.sync.dma_start(
                                out_ext[ot, :, q * 1024:(q + 1) * 1024],
                                out_sb[:, ot, q * 1024:(q + 1) * 1024])

    nc.compile()
    return nc


def _get_nc():
    if "nc" not in _CACHE:
        _CACHE["nc"] = _build_nc()
    return _CACHE["nc"]


def make_in_maps(x, Wq, Wkv, Wout, bout):
    import ml_dtypes
    bf16 = ml_dtypes.bfloat16
    xf = np.asarray(x, dtype=np.float32).reshape(B, C, N)
    Wq_r = np.asarray(Wq, np.float32).T.reshape(2, 128, INNER)
    Wk_r = np.asarray(Wkv, np.float32)[:INNER].T.reshape(2, 128, INNER)
    Wv_r = np.asarray(Wkv, np.float32)[INNER:].reshape(4, 128, QD)
    Wo_r = np.asarray(Wout, np.float32).T.reshape(4, 128, QD)
    bout_r = np.ascontiguousarray(
        np.asarray(bout, np.float32).reshape(2, 128).transpose(1, 0))
    Wq_r = Wq_r.transpose(1, 0, 2).astype(bf16)    # [128, 2, 512]
    Wk_r = Wk_r.transpose(1, 0, 2).astype(bf16)
    Wv_r = Wv_r.transpose(1, 0, 2).astype(bf16)    # [128, 4, 256]
    Wo_r = Wo_r.transpose(1, 0, 2).astype(bf16)
    maps = []
    xt_groups = []
    for g in range(2):
        Xg = xf[g * 8:(g + 1) * 8].reshape(QD, N)
        # [128 part = m%128, mc, 256 f] m-major full-group transpose
        xt_groups.append(
            Xg.T.reshape(MTF, 128, QD).transpose(1, 0, 2).astype(bf16))
    for i in range(N_CORES):
        g, s = divmod(i, 4)
        Xg = xf[g * 8:(g + 1) * 8].reshape(QD, N)
        xs = Xg[:, s * NSH:(s + 1) * NSH]
        # [(4b x 32c) part, cc, m] natural shard
        xs_n = xs.reshape(2, 128, NSH).transpose(1, 0, 2).astype(bf16)
        maps.append({
            "xT": xt_groups[g], "x": xs_n,
            "Wq": Wq_r, "Wk": Wk_r, "Wv": Wv_r, "Wout": Wo_r,
            "bout": bout_r,
        })
    return maps


def gather_out(results):
    out = np.empty((B, C, N), dtype=np.float32)
    for i in range(N_CORES):
        g, s = divmod(i, 4)
        r = np.asarray(results[i]["out"], np.float32).reshape(2, 4, C, NSH)
        for ot in range(2):
            out[g * 8 + ot * 4:g * 8 + (ot + 1) * 4, :,
                s * NSH:(s + 1) * NSH] = r[ot]
    return out.reshape(B, C, HS, WS)


def run_sharded(in_maps, **kw):
    from concourse.bass_utils import run_bass_kernel_spmd
    nc = _get_nc()
    return run_bass_kernel_spmd(nc, in_maps, list(range(N_CORES)), **kw)


def kernel(x, Wq, Wkv, Wout, bout):
    in_maps = make_in_maps(x, Wq, Wkv, Wout, bout)
    res = run_sharded(in_maps)
    return gather_out(res.results)


if __name__ == "__main__":
    nc = _get_nc()
    print("built + compiled OK")


# revision 24
# speedup vs baseline: 1.0494x; 1.0172x over previous
"""Trainium2 Bass kernel for nn_AttentionBlock (b=16, c=32, 128x128 spatial,
heads=8, dim_head=64).

Sharding: 4 spatial shards x 2 batch groups across 8 NeuronCores, with the
Gram matrix S = X X^T computed REDUNDANTLY on every core over the full
group N=16384 (instead of partial-S + AllReduce). The ncfw collective path
costs ~81us serial on this runtime (46us barrier + trigger pickup + 2 ARs)
and re-throttles the PE HAM clock during the idle wait; replicating S costs
~25us of extra PE streaming and removes all cross-core communication.

Algebraic structure (per core, X = group x reshaped [256=(8b x 32c), 16384]):
  S    = X X^T                  (Gram matrix, [256, 256]; m-reduction on PE;
                                 only S[0:128,:] and S[128:,128:] computed,
                                 S[128:,0:128] restored by PE transpose)
  sim_h = Wq_h (S/8) Wk_h^T     (tiny; q/k never materialized)
  attn = softmax(sim)           (local -- S is complete, no reduce needed)
  At[f=(h,j), o] = sum_i attn[h][i, j] Wout[o, h*64+i]
  Mt[c, o] = sum_f Wv[f, c] At[f, o]
  out[o, m] = sum_c Mt[c, o] X[c, m] + bout[o]   (own m-shard of 4096 only)

Perf notes: xT pieces go down the two HWDGE queues (sync/scalar, first
descriptor ~7us after NEFF start; gpsimd SWDGE costs ~1.4us/dma_start and
doesn't fire until ~11us, so it only carries the late-needed x/weights).
A dummy matmul keyed on the exp() output keeps the PE HAM clock at 8/8
through the softmax gap so the final gemm runs at 2.4GHz. Output is stored
bf16 (host upcasts) to halve the drain.
"""

import numpy as np

N_CORES = 8
B, C, HS, WS = 16, 32, 128, 128
N = HS * WS              # 16384
NSH = N // 4             # 4096 per-core spatial shard
H = 8
DH = 64
QD = H * C               # 256
INNER = H * DH           # 512
SCALE = DH ** -0.5       # 0.125
MTF = N // 128           # 128 m-chunks of 128 over the FULL group
NWARM = 40               # PE prewarm dummy matmuls (span the DMA lead time)
# xT piece schedule: (start_chunk, n_chunks, engine_idx) with engines
# [sync, scalar, gpsimd]. 512KB pieces (4KB/partition descriptors) keep
# per-piece latency ~3.5us; sync/scalar (first descriptor ~8.5us) carry
# the early pieces, gpsimd (SWDGE, first descriptor ~11.6us) carries
# pieces the PE only needs 16+ chunks in, so no queue's delivery lags
# S's ~1.4us/piece consumption.
XT_PIECES = [
    (0, 8, 0), (8, 8, 1), (16, 8, 0), (24, 8, 1), (32, 8, 2),
    (40, 8, 0), (48, 8, 1), (56, 8, 2), (64, 8, 0), (72, 8, 1),
    (80, 8, 2), (88, 8, 0), (96, 8, 1), (104, 8, 2), (112, 8, 0),
    (120, 8, 1),
]

_CACHE = {}


def _build_nc():
    import concourse.bacc as bacc
    import concourse.mybir as mybir
    import concourse.tile as tile
    from concourse.masks import make_identity
    from contextlib import ExitStack

    f32 = mybir.dt.float32
    bf16 = mybir.dt.bfloat16

    nc = bacc.Bacc("TRN2", target_bir_lowering=False, debug=False,
                   num_devices=N_CORES)

    # all inputs partition-major, pre-cast to bf16 on host so every DMA
    # descriptor is >=4KB contiguous and no cast-DMA is needed
    xt_ext = nc.dram_tensor("xT", [128, MTF, QD], bf16, kind="ExternalInput")
    x_ext = nc.dram_tensor("x", [128, 2, NSH], bf16, kind="ExternalInput")
    wq_ext = nc.dram_tensor("Wq", [128, 2, INNER], bf16, kind="ExternalInput")
    wk_ext = nc.dram_tensor("Wk", [128, 2, INNER], bf16, kind="ExternalInput")
    wv_ext = nc.dram_tensor("Wv", [128, 4, QD], bf16, kind="ExternalInput")
    wo_ext = nc.dram_tensor("Wout", [128, 4, QD], bf16, kind="ExternalInput")
    bout_ext = nc.dram_tensor("bout", [128, 2], f32, kind="ExternalInput")
    out_ext = nc.dram_tensor("out", [2, 128, NSH], bf16, kind="ExternalOutput")

    with tile.TileContext(nc) as tc:
        with ExitStack() as ctx:
            persist = ctx.enter_context(tc.tile_pool(name="persist", bufs=1))

            # identity first on gpsimd: ready ~6us, feeds the PE prewarm
            ident = persist.tile([128, 128], bf16, tag="ident")
            make_identity(nc, ident[:])

            # ---- loads: xT pieces per XT_PIECES over all 3 DMA queues;
            # weights ride behind gpsimd's pieces, x behind sync/scalar's.
            xt = persist.tile([128, MTF, QD], bf16, tag="xt")
            engs = [nc.sync, nc.scalar, nc.gpsimd]
            for (c0, nch, ei) in XT_PIECES:
                engs[ei].dma_start(
                    xt[:, c0:c0 + nch, :], xt_ext[:, c0:c0 + nch, :])

            wqT = persist.tile([128, 2, INNER], bf16, tag="wqT")
            nc.gpsimd.dma_start(wqT[:], wq_ext[:])
            wkT = persist.tile([128, 2, INNER], bf16, tag="wkT")
            nc.gpsimd.dma_start(wkT[:], wk_ext[:])
            wv_bf = persist.tile([128, 4, QD], bf16, tag="wv_bf")
            nc.gpsimd.dma_start(wv_bf[:], wv_ext[:])
            woT = persist.tile([128, 4, QD], bf16, tag="woT")
            nc.gpsimd.dma_start(woT[:], wo_ext[:])
            bout_sb = persist.tile([128, 2], f32, tag="bout_sb")
            nc.gpsimd.dma_start(bout_sb[:], bout_ext[:])
            x_bf = persist.tile([128, 2, NSH], bf16, tag="x_bf")
            xq = [nc.sync, nc.scalar, nc.gpsimd, nc.sync]
            for cc in range(2):
                for hh in range(2):
                    xq[cc * 2 + hh].dma_start(
                        x_bf[:, cc, hh * 2048:(hh + 1) * 2048],
                        x_ext[:, cc, hh * 2048:(hh + 1) * 2048])

            S_bf = [persist.tile([128, 256], bf16, tag=f"S{c}",
                                 name=f"S{c}") for c in range(2)]
            T1 = [persist.tile([128, 512], bf16, tag=f"T1{c}",
                               name=f"T1{c}") for c in range(2)]

            with tc.tile_pool(name="S0ps", bufs=1, space="PSUM") as S0pool, \
                 tc.tile_pool(name="S11ps", bufs=1, space="PSUM") as S11pool, \
                 tc.tile_pool(name="tpps", bufs=1, space="PSUM") as tppool, \
                 tc.tile_pool(name="t1ps", bufs=2, space="PSUM") as t1ps, \
                 tc.tile_pool(name="wrm1", bufs=1, space="PSUM") as wrm1, \
                 tc.tile_pool(name="simps", bufs=1, space="PSUM") as simpool:

                # PE prewarm: dummy matmuls on the identity tile bridge the
                # ~5us DMA lead time so the HAM clock is already 8/8 (2.4GHz)
                # when the first xT piece lands
                wm = wrm1.tile([128, 256], f32, tag="wm")
                for _ in range(NWARM):
                    nc.tensor.matmul(wm[:, 0:128], ident[:], ident[:],
                                     start=True, stop=True)

                # S = X X^T over the FULL group m range. Symmetric: compute
                # rows 0-127 x all cols (S0) and the S11 block; S10 = S01^T.
                # Two accumulation groups in two separate PSUM banks.
                S0_ps = S0pool.tile([128, 256], f32, tag="S0ps")
                S11_ps = S11pool.tile([128, 128], f32, tag="S11ps")
                for mc in range(MTF):
                    nc.tensor.matmul(
                        S0_ps[:], xt[:, mc, 0:128], xt[:, mc, :],
                        start=(mc == 0), stop=(mc == MTF - 1))
                    nc.tensor.matmul(
                        S11_ps[:], xt[:, mc, 128:256], xt[:, mc, 128:256],
                        start=(mc == 0), stop=(mc == MTF - 1))
                nc.vector.tensor_copy(S_bf[0][:], S0_ps[:])
                nc.scalar.copy(S_bf[1][:, 128:256], S11_ps[:])
                tp_ps = tppool.tile([128, 128], bf16, tag="tpps")
                nc.tensor.transpose(tp_ps[:], S_bf[0][:, 128:256], ident[:])
                # junk burst keyed on S_bf[0]: keeps PE activity dense while
                # the S10 reconstruction copies run (HAM re-throttles on low
                # duty, not just full idle)
                for _ in range(12):
                    nc.tensor.matmul(wm[:, 128:256], S_bf[0][:, 0:128],
                                     S_bf[0][:, 0:128], start=True, stop=True)
                nc.vector.tensor_copy(S_bf[1][:, 0:128], tp_ps[:])

                # T1 = S Wk^T  (S symmetric; accumulate over c2 chunks)
                for c1 in range(2):
                    t1p = t1ps.tile([128, 512], f32, tag="t1p")
                    for c2 in range(2):
                        nc.tensor.matmul(
                            t1p[:],
                            S_bf[c2][:, c1 * 128:(c1 + 1) * 128],
                            wkT[:, c2, :],
                            start=(c2 == 0), stop=(c2 == 1))
                    if c1 == 0:
                        nc.vector.tensor_copy(T1[c1][:], t1p[:])
                    else:
                        nc.scalar.copy(T1[c1][:], t1p[:])

                # sim_h = Wq_h T1_h  (diagonal 128-blocks)
                sim_ps = simpool.tile([128, 512], f32, tag="simps")
                for hp in range(4):
                    hs = slice(hp * 128, (hp + 1) * 128)
                    for c1 in range(2):
                        nc.tensor.matmul(
                            sim_ps[:, hs], wqT[:, c1, hs], T1[c1][:, hs],
                            start=(hp == 0 and c1 == 0),
                            stop=(hp == 3 and c1 == 1))

                # fused extract+softmax-numerator: expt = exp(SCALE*sim)
                # straight from PSUM, with the row-sum accumulated for free
                # (logits are O(0.5) so exp needs no max-shift). Separate
                # tiles per diagonal block hp so downstream consumers (and
                # the PE keep-warm matmul) fire per-block instead of waiting
                # for the whole extract phase.
                expt = [persist.tile([128, 64], f32, tag=f"expt{hp}",
                                     name=f"expt{hp}") for hp in range(4)]
                sums = [persist.tile([128, 1], f32, tag=f"sums{hp}",
                                     name=f"sums{hp}") for hp in range(4)]
                Exp = mybir.ActivationFunctionType.Exp
                for hp in range(4):
                    nc.scalar.activation(
                        expt[hp][0:64, :],
                        sim_ps[0:64, hp * 128:hp * 128 + 64], Exp,
                        scale=SCALE, accum_out=sums[hp][0:64, :])
                    nc.scalar.activation(
                        expt[hp][64:128, :],
                        sim_ps[64:128, hp * 128 + 64:hp * 128 + 128], Exp,
                        scale=SCALE, accum_out=sums[hp][64:128, :])

            # ---- softmax -> At -> Mt -> final gemm -> out stores ----
            with tc.tile_pool(name="smx", bufs=1) as smx, \
                 tc.tile_pool(name="warm", bufs=1, space="PSUM") as warmpool, \
                 tc.tile_pool(name="aps", bufs=2, space="PSUM") as aps, \
                 tc.tile_pool(name="mps", bufs=1, space="PSUM") as mps, \
                 tc.tile_pool(name="ops", bufs=3, space="PSUM") as ops:
                At = [persist.tile([128, 256], bf16, tag=f"At{fc}",
                                   name=f"At{fc}") for fc in range(4)]
                Mt = [persist.tile([128, 256], bf16, tag=f"Mt{cc}",
                                   name=f"Mt{cc}") for cc in range(2)]
                out_sb = persist.tile([128, 2, NSH], bf16, tag="out_sb")

                # junk bursts keyed on successive chain tiles: each fires as
                # its tile is produced, keeping the PE HAM activity window
                # dense through the softmax chain so the tail stays at 2.4GHz
                warm_ps = warmpool.tile([128, 64], f32, tag="warm")
                warm_b = warmpool.tile([128, 256], f32, tag="warmb")
                for _ in range(12):
                    nc.tensor.matmul(warm_ps[0:64, :], expt[0][:],
                                     expt[0][:], start=True, stop=True)
                for _ in range(12):
                    nc.tensor.matmul(warm_ps[0:64, :], expt[2][:],
                                     expt[2][:], start=True, stop=True)

                # per-block softmax finish + At, pipelined across engines:
                # recip(hp) -> attn(hp) -> At matmuls(hp) while hp+1's
                # extract is still running on the scalar engine.
                # At[fc][(parity,j), o] = sum_i attn[h][i, j] WoutT[f, o]
                for fc in range(4):
                    rsum = smx.tile([128, 1], f32, tag=f"rsum{fc}",
                                    name=f"rsum{fc}")
                    nc.vector.reciprocal(rsum[:], sums[fc][:])
                    attn_bf = smx.tile([128, 64], bf16, tag=f"attn{fc}",
                                       name=f"attn{fc}")
                    nc.vector.tensor_mul(attn_bf[:], expt[fc][:],
                                         rsum[:].broadcast_to([128, 64]))
                    ap_t = aps.tile([128, 256], f32, tag="ap_t")
                    for parity in range(2):
                        ps = slice(parity * 64, (parity + 1) * 64)
                        nc.tensor.matmul(ap_t[ps, :], attn_bf[ps, :],
                                         woT[ps, fc, :],
                                         start=True, stop=True)
                    if fc % 2 == 0:
                        nc.vector.tensor_copy(At[fc][:], ap_t[:])
                    else:
                        nc.scalar.copy(At[fc][:], ap_t[:])
                    if fc == 1:
                        for _ in range(12):
                            nc.tensor.matmul(warm_b[:], At[0][:, 0:128],
                                             At[0][:], start=True, stop=True)

                # Mt[c, o] = sum_f Wv[f, c] At[f, o]
                mp = mps.tile([128, 512], f32, tag="mp")
                for cchunk in range(2):
                    cs = slice(cchunk * 128, (cchunk + 1) * 128)
                    for fc in range(4):
                        nc.tensor.matmul(
                            mp[:, cchunk * 256:(cchunk + 1) * 256],
                            wv_bf[:, fc, cs], At[fc][:],
                            start=(cchunk == 0 and fc == 0),
                            stop=(cchunk == 1 and fc == 3))
                for cchunk in range(2):
                    eng_copy = (nc.vector.tensor_copy if cchunk == 0
                                else nc.scalar.copy)
                    eng_copy(Mt[cchunk][:],
                             mp[:, cchunk * 256:(cchunk + 1) * 256])
                for _ in range(8):
                    nc.tensor.matmul(warm_b[:], Mt[0][:, 0:128],
                                     Mt[0][:], start=True, stop=True)

                # out[o, m] = sum_c Mt[c, o] X[c, m] + bout; store per 1024-m
                for ot in range(2):
                    os_ = slice(ot * 128, (ot + 1) * 128)
                    for pc in range(8):
                        op_t = ops.tile([128, 512], f32, tag="op_t")
                        for cc in range(2):
                            nc.tensor.matmul(op_t[:], Mt[cc][:, os_],
                                             x_bf[:, cc, pc * 512:(pc + 1) * 512],
                                             start=(cc == 0), stop=(cc == 1))
                        dst = out_sb[:, ot, pc * 512:(pc + 1) * 512]
                        if pc % 2 == 0:
                            nc.vector.tensor_scalar_add(
                                dst, op_t[:], bout_sb[:, ot:ot + 1])
                        else:
                            nc.scalar.activation(
                                dst, op_t[:],
                                mybir.ActivationFunctionType.Identity,
                                bias=bout_sb[:, ot:ot + 1])
                            q = pc // 2
                            nc.sync.dma_start(
                                out_ext[ot, :, q * 1024:(q + 1) * 1024],
                                out_sb[:, ot, q * 1024:(q + 1) * 1024])

    nc.compile()
    return nc


def _get_nc():
    if "nc" not in _CACHE:
        _CACHE["nc"] = _build_nc()
    return _CACHE["nc"]


def make_in_maps(x, Wq, Wkv, Wout, bout):
    import ml_dtypes
    bf16 = ml_dtypes.bfloat16
    xf = np.asarray(x, dtype=np.float32).reshape(B, C, N)
    Wq_r = np.asarray(Wq, np.float32).T.reshape(2, 128, INNER)
    Wk_r = np.asarray(Wkv, np.float32)[:INNER].T.reshape(2, 128, INNER)
    Wv_r = np.asarray(Wkv, np.float32)[INNER:].reshape(4, 128, QD)
    Wo_r = np.asarray(Wout, np.float32).T.reshape(4, 128, QD)
    bout_r = np.ascontiguousarray(
        np.asarray(bout, np.float32).reshape(2, 128).transpose(1, 0))
    Wq_r = Wq_r.transpose(1, 0, 2).astype(bf16)    # [128, 2, 512]
    Wk_r = Wk_r.transpose(1, 0, 2).astype(bf16)
    Wv_r = Wv_r.transpose(1, 0, 2).astype(bf16)    # [128, 4, 256]
    Wo_r = Wo_r.transpose(1, 0, 2).astype(bf16)
    maps = []
    xt_groups = []
    for g in range(2):
        Xg = xf[g * 8:(g + 1) * 8].reshape(QD, N)
        # [128 part = m%128, mc, 256 f] m-major full-group transpose
        xt_groups.append(
            Xg.T.reshape(MTF, 128, QD).transpose(1, 0, 2).astype(bf16))
    for i in range(N_CORES):
        g, s = divmod(i, 4)
        Xg = xf[g * 8:(g + 1) * 8].reshape(QD, N)
        xs = Xg[:, s * NSH:(s + 1) * NSH]
        # [(4b x 32c) part, cc, m] natural shard
        xs_n = xs.reshape(2, 128, NSH).transpose(1, 0, 2).astype(bf16)
        maps.append({
            "xT": xt_groups[g], "x": xs_n,
            "Wq": Wq_r, "Wk": Wk_r, "Wv": Wv_r, "Wout": Wo_r,
            "bout": bout_r,
        })
    return maps


def gather_out(results):
    out = np.empty((B, C, N), dtype=np.float32)
    for i in range(N_CORES):
        g, s = divmod(i, 4)
        r = np.asarray(results[i]["out"], np.float32).reshape(2, 4, C, NSH)
        for ot in range(2):
            out[g * 8 + ot * 4:g * 8 + (ot + 1) * 4, :,
                s * NSH:(s + 1) * NSH] = r[ot]
    return out.reshape(B, C, HS, WS)


def run_sharded(in_maps, **kw):
    from concourse.bass_utils import run_bass_kernel_spmd
    nc = _get_nc()
    return run_bass_kernel_spmd(nc, in_maps, list(range(N_CORES)), **kw)


def kernel(x, Wq, Wkv, Wout, bout):
    in_maps = make_in_maps(x, Wq, Wkv, Wout, bout)
    res = run_sharded(in_maps)
    return gather_out(res.results)


if __name__ == "__main__":
    nc = _get_nc()
    print("built + compiled OK")
